# revision 54
# baseline (speedup 1.0000x reference)
"""Bahdanau additive attention Trainium2 Bass kernel (v3, bf16 + XBAR).

Reference (per batch b):
    U = key @ W_encoder.T                  # [S, A]
    V = q @ W_decoder.T                    # [A]
    score = tanh(U + V) @ v[0]             # [S]
    w = softmax(score)                     # [S]
    context = w @ key                      # [KD]

Sharding: data-parallel over batch across 8 NeuronCores (4 batches/core),
weights replicated.  All matmuls in bf16 (output rel err ~3e-4, tolerance
2e-2).

Dataflow per core:
  - key / W_encoder / W_decoder are cast fp32->bf16 by gpsimd DMA into
    DRAM staging tiles (contiguous, so ~16 descriptors per MB), then the
    DMA XBAR transpose unit (dma_start(transpose=True), 2-byte dtypes,
    ~14ns per 16x128 tile) lands them in SBUF already transposed:
       ktx[k', t, s]   = key[chunk_s + s, t*128 + k']     [128, 8, 512]
       wetx[m][k',t,a'] = We[m*128 + a', t*128 + k']      [128, 8, 128]
    This keeps all transposes off the PE (the v1 fp32r PE transposes
    cost ~109us/core at 4 cycles/row).
  - U^T a-tiles [128 a', 512 s] accumulate over 8 k-tiles on PE
    (stationary wetx[m][:, t, :], moving ktx[:, t, :]).
  - tanh(U+V) on ACT (V as per-partition bias), bf16 out.
  - score row = vcols.T @ th on PE, interleaved with the next chunk's U
    matmuls so the PE never waits on ACT.
  - e = exp(score) on ACT (no max subtraction: |score| <= sum|v| ~ 26),
    accum_out collects the chunk sum; gpsimd broadcasts e to 128
    partitions.
  - context accumulates on DVE: prod = ktx * e_bcast per k-tile, one 3D
    free-axis reduce_sum to [128, nkt] per chunk, added into a per-batch
    accumulator -- the PE never re-streams key.  (tensor_tensor_reduce
    would fuse this, but that ISA op faults on real hardware.)
  - One flat stream of work items with prefetch distance 2; the last
    chunk is split into 4x128 subchunks so the final score/exp/context
    drain (the only non-overlapped tail) is short.  Weight quarters load
    alternately We/Wd into separate tiles (a shared tile would serialize
    the loads via tile-level WAW deps) so chunk 0 can transpose We row m
    and compute the V-bias slice m just in time per a-tile.
Per-core PE work: U 218us + score 27us (cost model) + ~13us transposes.
"""
import sys
sys.path.insert(0, "/opt/trn_rl_repo")

from contextlib import ExitStack

import numpy as np

import concourse.bass as bass
import concourse.tile as tile
from concourse import bacc, bass_isa, masks, mybir

dt = mybir.dt
AF = mybir.ActivationFunctionType
ALU = mybir.AluOpType

# Full problem shape
B, S, KD, QD, AD = 32, 2048, 1024, 1024, 1024
N_CORES = 8
BS = B // N_CORES          # batches per core
SC = 512                   # s-chunk (columns per U matmul)


def build_kernel(nc, bs=BS, s=S, kd=KD, qd=QD, ad=AD, reps=1, dyn_reps=0):
    """Emit the per-core kernel into `nc` (a bacc.Bacc)."""
    f32, bf16 = dt.float32, dt.bfloat16
    nsc = s // SC            # s-chunks per batch
    nkt = kd // 128          # k-tiles
    nat = ad // 128          # a-tiles
    nqt = qd // 128          # q-tiles
    assert s % SC == 0 and kd % 128 == 0 and ad % 128 == 0 and qd % 128 == 0

    key_d = nc.dram_tensor("key", [bs, s, kd], f32, kind="ExternalInput").ap()
    q_d = nc.dram_tensor("q", [bs, qd], f32, kind="ExternalInput").ap()
    we_d = nc.dram_tensor("W_encoder", [ad, kd], f32, kind="ExternalInput").ap()
    wd_d = nc.dram_tensor("W_decoder", [ad, qd], f32, kind="ExternalInput").ap()
    v_d = nc.dram_tensor("v", [1, ad], f32, kind="ExternalInput").ap()
    out_d = nc.dram_tensor("out", [bs, kd], f32, kind="ExternalOutput").ap()

    with tile.TileContext(nc) as tc, ExitStack() as ctx:
        const = ctx.enter_context(tc.tile_pool(name="const", bufs=1))

        ident_f = const.tile([128, 128], f32, name="ident_f")
        masks.make_identity(nc, ident_f[:])
        ident_b = const.tile([128, 128], bf16, name="ident_b")
        nc.vector.tensor_copy(ident_b[:], ident_f[:])
        one_f = const.tile([1, 1], f32, name="one_f")
        nc.gpsimd.memset(one_f[:], 1.0)

        # WeT tiles: wetx[m][k', t, a'] = We[m*128 + a', t*128 + k'] (bf16)
        wetx = [const.tile([128, nkt, 128], bf16, name=f"wetx{m}")
                for m in range(nat)]
        # V bias per a-tile [128, bs] fp32; v columns [128, nat] bf16.
        vbias = [const.tile([128, bs], f32, name=f"vbias{m}")
                 for m in range(nat)]
        vcols = const.tile([128, nat], bf16, name="vcols")

        # ---------------- pools ----------------
        kstage = ctx.enter_context(tc.tile_pool(name="kstage", bufs=5,
                                                space="DRAM"))
        ktpool = ctx.enter_context(tc.tile_pool(name="ktx", bufs=5))
        thpool = ctx.enter_context(tc.tile_pool(name="tanh", bufs=2))
        spool = ctx.enter_context(tc.tile_pool(name="small", bufs=2))
        epool = ctx.enter_context(tc.tile_pool(name="ebc", bufs=3))
        pp_u = ctx.enter_context(tc.tile_pool(name="pp_u", bufs=4, space="PSUM"))
        pp_s = ctx.enter_context(tc.tile_pool(name="pp_s", bufs=2, space="PSUM"))
        pp_w = ctx.enter_context(tc.tile_pool(name="pp_w", bufs=1, space="PSUM"))

        def load_chunk(uid, b, s0, w):
            """key chunk fp32->bf16 cast into DRAM staging (contiguous, so
            only ~16 DMA descriptors), then XBAR transpose into SBUF."""
            kst = kstage.tile([w, kd], bf16, name=f"kst{uid}", tag="kst")
            nc.gpsimd.dma_start(kst[:], key_d[b, s0:s0 + w, :])
            # ktx[k', t, s] = key[s0 + s, t*128 + k']
            ktx = ktpool.tile([128, nkt, w], bf16, name=f"ktx{uid}",
                              tag="ktx")
            nc.sync.dma_start(ktx[:], kst[:], transpose=True)
            return ktx

        # ---------------- weight prep (once per core) ----------------
        # Weight rows are cast-loaded straight to SBUF (gpsimd DMA) and
        # transposed with bf16 PE transpose-mode matmuls (53ns per 128x128
        # block; the PE is idle during startup anyway).  Only the big key
        # tensor goes through the DRAM-staging + XBAR path.
        pre = {}
        with tc.tile_pool(name="wprep", bufs=1) as wprep:
            qn = wprep.tile([bs, qd], f32, name="qn")
            nc.sync.dma_start(qn[:], q_d)
            vrow = wprep.tile([1, ad], f32, name="vrow")
            nc.sync.dma_start(vrow[:], v_d)

            # Weight cast-loads in alternating two-row quarters: each is
            # 256 SWDGE descriptors (the ring holds 1536) into its OWN
            # tile (a shared destination tile would serialize the loads
            # through tile-level write-after-write deps), and the
            # alternation delivers We row m and Wd row m just before the
            # first chunk's U a-tile m / V-bias m need them.
            werq = [wprep.tile([128, 2, kd], bf16, name=f"werq{q}")
                    for q in range(nat // 2)]
            wdrq = [wprep.tile([128, 2, qd], bf16, name=f"wdrq{q}")
                    for q in range(nat // 2)]
            pre[0] = load_chunk("p0", 0, 0, SC)
            wev = we_d.rearrange("(m p) k -> p m k", p=128)
            wdv = wd_d.rearrange("(m p) k -> p m k", p=128)
            for q in range(nat // 2):
                nc.gpsimd.dma_start(werq[q][:], wev[:, 2 * q:2 * q + 2])
                nc.gpsimd.dma_start(wdrq[q][:], wdv[:, 2 * q:2 * q + 2])
            pre[1] = load_chunk("p1", 0, SC, SC)

            wdxt = [wprep.tile([128, nqt, 128], bf16, name=f"wdxt{m}",
                               tag=f"wdxt{m}") for m in range(nat)]
            qt = wprep.tile([128, nqt * bs], bf16, name="qt")

            # qT: [q', t*bs + b] bf16 via PE transpose-mode
            psq = pp_w.tile([128, nqt * bs], f32, name="psq", tag="pw")
            for t in range(nqt):
                nc.tensor.matmul(psq[:, t * bs:(t + 1) * bs],
                                 qn[:, t * 128:(t + 1) * 128],
                                 ident_f[:bs, :bs], is_transpose=True)
            nc.vector.tensor_copy(qt[:], psq[:])

            # v columns [128, nat]
            psvc = pp_w.tile([128, nat], f32, name="psvc", tag="pw")
            for m in range(nat):
                nc.tensor.matmul(psvc[:, m:m + 1],
                                 vrow[:, m * 128:(m + 1) * 128],
                                 one_f[:], is_transpose=True)
            nc.vector.tensor_copy(vcols[:], psvc[:])

            def emit_vprep_m(m):
                # Wd row-tile m: transpose (PE transpose-mode, bf16) then
                # V slice m = Wd[m-rows] @ q into vbias[m].
                pst = pp_s.tile([128, qd], bf16, name=f"wdt{m}",
                                tag="pss")
                for t in range(nqt):
                    nc.tensor.matmul(pst[:, t * 128:(t + 1) * 128],
                                     wdrq[m // 2][:, m % 2,
                                                  t * 128:(t + 1) * 128],
                                     ident_b[:], is_transpose=True)
                nc.vector.tensor_copy(wdxt[m][:], pst[:])
                psv = pp_w.tile([128, bs], f32, name=f"psv{m}", tag="pw")
                for t in range(nqt):
                    nc.tensor.matmul(psv[:], wdxt[m][:, t, :],
                                     qt[:, t * bs:(t + 1) * bs],
                                     start=(t == 0), stop=(t == nqt - 1))
                nc.vector.tensor_copy(vbias[m][:], psv[:])

            def emit_wetx(m):
                pst = pp_s.tile([128, kd], bf16, name=f"wet{m}", tag="pss")
                for t in range(nkt):
                    nc.tensor.matmul(pst[:, t * 128:(t + 1) * 128],
                                     werq[m // 2][:, m % 2,
                                                  t * 128:(t + 1) * 128],
                                     ident_b[:], is_transpose=True)
                nc.vector.tensor_copy(wetx[m][:], pst[:])

            # ---------------- main streaming loop ----------------
            # One flat stream of chunks (global index g = b*nsc + c) with
            # prefetch distance 2; batch epilogues are emitted when the
            # batch's last score drains, i.e. inside the NEXT batch's first
            # chunk, so no queue ever blocks on a batch boundary.
            def emit_epilogue(tagb, b, zparts, ctxa):
                z = spool.tile([1, 1], f32, name=f"z{tagb}", tag="z")
                nc.vector.reduce_sum(z[:], zparts[:],
                                     axis=mybir.AxisListType.X)
                rz = spool.tile([1, 1], f32, name=f"rz{tagb}", tag="rz")
                nc.vector.reciprocal(rz[:], z[:])
                rzb = spool.tile([128, 1], f32, name=f"rzb{tagb}",
                                 tag="rzb")
                nc.gpsimd.partition_broadcast(rzb[:], rz[:])
                cn = spool.tile([128, nkt], f32, name=f"cn{tagb}", tag="cn")
                nc.vector.tensor_scalar_mul(cn[:], ctxa[:], rzb[:])
                # out[k = t*128 + p] <- cn[p, t]: 128 strided descriptors
                # of 32B -- small enough to be cheap.
                nc.sync.dma_start(
                    out_d[b:b + 1, :]
                    .rearrange("o (t p) -> p (o t)", p=128),
                    cn[:])

            def emit_body(rep, first=False):
                # Work items (b, zi, s0, w): one per 512-wide chunk, except
                # the very last chunk which is split into 4x128 so its
                # score/exp/context drain chain (the only non-overlapped
                # tail) is 4x shorter.
                items = []
                for b in range(bs):
                    for c in range(nsc):
                        if b == bs - 1 and c == nsc - 1:
                            for j in range(4):
                                items.append((b, c + j, c * SC + j * 128,
                                              128))
                        else:
                            items.append((b, c, c * SC, SC))
                ng = len(items)
                state = {}    # b -> (zparts, ctxa)
                ktx_by_g = {}
                pend = None   # (g, ths, pss, next_m)

                def finish_chunk(g, pss):
                    b, zi, s0, w = items[g]
                    tagb = f"r{rep}b{b}"
                    zparts, ctxa = state[b]
                    ktx_g = ktx_by_g.pop(g)
                    erow = spool.tile([1, w], bf16,
                                      name=f"erow{tagb}z{zi}", tag="erow")
                    nc.scalar.activation(erow[:], pss[:], AF.Exp,
                                         accum_out=zparts[:, zi:zi + 1])
                    ebc = epool.tile([128, w], bf16,
                                     name=f"ebc{tagb}z{zi}", tag="ebc")
                    nc.gpsimd.partition_broadcast(ebc[:], erow[:])
                    # context partials: prod[k', t, s] = ktx * e, then one
                    # 3D free-axis reduce to [128, nkt], accumulated into
                    # ctxa.  (tensor_tensor_reduce would fuse this, but
                    # that ISA op faults on hardware.)
                    prod = epool.tile([128, nkt, w], bf16,
                                      name=f"prod{tagb}z{zi}", tag="prod")
                    for t in range(nkt):
                        nc.vector.tensor_tensor(prod[:, t, :],
                                                ktx_g[:, t, :], ebc[:],
                                                op=ALU.mult)
                    cpart = spool.tile([128, nkt], f32,
                                       name=f"cp{tagb}z{zi}", tag="cpart")
                    nc.vector.reduce_sum(cpart[:], prod[:],
                                         axis=mybir.AxisListType.X)
                    if zi == 0:
                        nc.vector.tensor_copy(ctxa[:], cpart[:])
                    else:
                        nc.vector.tensor_tensor(ctxa[:], ctxa[:], cpart[:],
                                                op=ALU.add)
                    if g == ng - 1 or items[g + 1][0] != b:
                        emit_epilogue(tagb, b, zparts, ctxa)
                        del state[b]

                def start_score(g, ths):
                    nonlocal pend
                    b, zi, s0, w = items[g]
                    pss = pp_s.tile([1, w], f32,
                                    name=f"pssr{rep}b{b}z{zi}", tag="pss")
                    nc.tensor.matmul(pss[:], vcols[:, 0:1], ths[0][:],
                                     start=True, stop=False,
                                     skip_group_check=True)
                    pend = (g, ths, pss, 1)

                def step_score():
                    nonlocal pend
                    if pend is None:
                        return
                    g, ths, pss, m = pend
                    nc.tensor.matmul(pss[:], vcols[:, m:m + 1],
                                     ths[m][:], start=False,
                                     stop=(m == nat - 1),
                                     skip_group_check=True)
                    if m == nat - 1:
                        pend = None
                        finish_chunk(g, pss)
                    else:
                        pend = (g, ths, pss, m + 1)

                nzp = max(it[1] for it in items) + 1
                for g in range(ng):
                    b, zi, s0, w = items[g]
                    tagb = f"r{rep}b{b}"
                    if zi == 0:
                        state[b] = (
                            spool.tile([1, nzp], f32, name=f"zp{tagb}",
                                       tag="zparts"),
                            spool.tile([128, nkt], f32, name=f"ctxa{tagb}",
                                       tag="ctxa"))
                    if first and g in pre:
                        ktx_by_g[g] = pre.pop(g)
                    elif g < 2 and not first:
                        ktx_by_g[g] = load_chunk(f"{tagb}z{zi}", b, s0, w)
                    if g + 2 < ng and (not first or (g + 2) not in pre):
                        b2, z2, s02, w2 = items[g + 2]
                        ktx_by_g[g + 2] = load_chunk(
                            f"r{rep}b{b2}z{z2}", b2, s02, w2)
                    ktx = ktx_by_g[g]

                    def emit_u(m):
                        psu = pp_u.tile([128, w], f32,
                                        name=f"psu{tagb}z{zi}m{m}",
                                        tag="psu")
                        for t in range(nkt):
                            nc.tensor.matmul(
                                psu[:], wetx[m][:, t, :], ktx[:, t, :],
                                start=(t == 0), stop=(t == nkt - 1))
                        return psu

                    def emit_tanh(m, psu):
                        th = thpool.tile([128, w], bf16,
                                         name=f"th{tagb}z{zi}m{m}",
                                         tag=f"th{m}")
                        nc.scalar.activation(th[:], psu[:], AF.Tanh,
                                             bias=vbias[m][:, b:b + 1])
                        ths.append(th)

                    ths = []
                    if first and g == 0:
                        # Special chunk 0: We transpose, V-bias slice, U
                        # and tanh just-in-time per a-tile, paced by the
                        # alternating We/Wd quarter loads.
                        for m in range(nat):
                            emit_wetx(m)
                            emit_vprep_m(m)
                            emit_tanh(m, emit_u(m))
                    else:
                        for m in range(nat):
                            emit_tanh(m, emit_u(m))
                            step_score()
                    start_score(g, ths)
                    if g == ng - 1:
                        while pend is not None:
                            step_score()

            if dyn_reps:
                with tc.For_i(0, dyn_reps, 1):
                    emit_body(0)
            else:
                for rep in range(reps):
                    emit_body(rep, first=(rep == 0))
    return nc


_CACHE = {}


def _get_compiled(cfg):
    if cfg not in _CACHE:
        # 1536-descriptor SWDGE ring (default 1024): keeps the startup
        # burst of weight-quarter cast-loads (256 desc each) plus key
        # chunk stages from overflowing the ring, which would head-block
        # the Pool sequencer.
        nc = bacc.Bacc("TRN2", target_bir_lowering=False, debug=False,
                       dynamic_dma_scratch_size=24576)
        build_kernel(nc, *cfg)
        nc.compile()
        _CACHE[cfg] = nc
    return _CACHE[cfg]


def kernel(**inputs):
    from concourse.bass_utils import run_bass_kernel_spmd

    key = np.asarray(inputs["key"], dtype=np.float32)
    q = np.asarray(inputs["q"], dtype=np.float32)
    we = np.asarray(inputs["W_encoder"], dtype=np.float32)
    wd = np.asarray(inputs["W_decoder"], dtype=np.float32)
    v = np.asarray(inputs["v"], dtype=np.float32)

    nc = _get_compiled((BS, S, KD, QD, AD, 1))
    in_maps = []
    for cidx in range(N_CORES):
        sl = slice(cidx * BS, (cidx + 1) * BS)
        in_maps.append({
            "key": key[sl], "q": q[sl],
            "W_encoder": we, "W_decoder": wd, "v": v,
        })
    res = run_bass_kernel_spmd(nc, in_maps, list(range(N_CORES))).results
    return np.concatenate([r["out"] for r in res], axis=0)
